# revision 31
# baseline (speedup 1.0000x reference)
"""Trainium2 Bass kernel for nn_CViTFlow (cross-attention ViT flow block).

Math (per the module):
  two token streams x1,x2 [B,T,256] viewed as [B,256,48,48] images.
  6 branches (q1,k1,v1,q2,k2,v2): depthwise3x3 -> BN(eval) -> 1x1 conv -> Linear.
  o1 = softmax(-(q1 k2^T / 16)) v2 + q1 ;  o2 = softmax(-(q2 k1^T / 16)) v1 + q2
  both reshaped [B,H,T,DH] -> [B,T,256] with a plain (head-major) reshape.

Kernel strategy (v2 — single fused software pipeline):
  * Host folds BN + 1x1conv + Linear into one 256x256 matrix W and bias c per
    branch, then folds the depthwise 3x3 into 9 "tap" matrices, so a branch is
    9 shifted matmuls accumulated in PSUM.  8 cores = (map, batch, head-quad);
    no collectives.  (Same host-side prep as v1.)
  * The whole kernel is ONE pipeline paced by the ScalarE exp stream (the hard
    floor: 162 ACTIVATEs x [128,1024] ~= 216us).  No serial branch phase:
    - upfront: k-branch tiles interleaved with l-tile-0 scores+exp as kT
      chunks become available (exp starts ~12us in),
    - the main j-loop (one step per (l-tile, t-chunk)) carries v-branch,
      transposes, and q-branch tiles 2-4 as PE filler items, plus deferred AV.
  * Scores: ONE matmul per j (K=128, N=1024) using a block-diagonal expanded
    rhs qTx [128, 4*256] (head h's q strip in rows 32h, zeros elsewhere); the
    zero rows select each head's 32 dims out of the full-128 contraction.
    1 LDWEIGHTS (vs 4 strip loads) and a clean 2-bank PSUM tile per j.
  * Score tiles double-buffered (2 x 2 banks) -> exp(j) never WARs scores(j+1)
    and the exp chain runs back-to-back on ScalarE.
  * AV: per (j, head) matmul with M=33 weights [v(32) | ones] (LDWEIGHTS cost
    scales with columns: 33 not 64); ones column accumulates the softmax
    denominator.  AV is DEFERRED (emission gated on its vaug chunk having been
    emitted -> deadlock-free by construction) and catches up 2/step.
  * Finalize per l-tile: 4 denominator row-copies -> ONE [4,256] f32r
    reciprocal -> ONE mask4 broadcast matmul (K=4 -> all 128 rows) -> 4 mult +
    4 residual-add, then the output slice DMAs out.  Spread one substep per
    loop step.
  * PSUM (8 banks): sc 2x2 + avout 2x1 + scratch(branch/transpose/bc) 2x1.
"""

import numpy as np

B = 2
T = 2304
DIM = 256
HEADS = 8
DH = 32
HW = 48
EPS = 1e-5
P = 128
N_CORES = 8

# t-tiles for the branch phase: row-aligned in the 48x48 image (10/8 rows)
T_TILES = [(0, 480, 0, 10), (480, 480, 10, 10), (960, 480, 20, 10),
           (1440, 480, 30, 10), (1920, 384, 40, 8)]
# image-row bands for the input DMAs: tile k's taps read rows r0..r0+nr+1,
# so band boundaries at row cuts let each branch tile start as soon as the
# bands covering its rows have landed
DMA_BANDS = [(0, 12), (12, 22), (22, 32), (32, 42), (42, 50)]
NL = 256
N_LT = T // NL  # 9
N_TCH = T // P  # 18 t-chunks of 128 for scores/AV
NJ = N_LT * N_TCH  # 162 pipeline steps
# chunks fully covered after branch t-tile i has drained
TILE_CHUNKS = [(0, 3), (3, 7), (7, 11), (11, 15), (15, 18)]
ET_BUFS = 36  # et ring: AV may lag exp by up to ET_BUFS-2 steps

_PROGRAM = None  # cached Bass program
_last_in_maps = None  # stashed per-core input maps (for external profiling runs)


def _build_program(debug=False):
    """Build the SPMD Bass/Tile program (identical for all 8 cores)."""
    import sys
    import types
    from contextlib import ExitStack

    # the rust AP-lowering path does `import log` when it encounters a
    # not-yet-bound virtual tensor; the module doesn't exist in this image
    if "log" not in sys.modules:
        _log = types.ModuleType("log")
        for _fn in ("debug", "info", "warning", "warn", "error", "critical",
                    "exception"):
            setattr(_log, _fn, lambda *a, **k: None)
        sys.modules["log"] = _log

    import concourse.bacc as bacc
    import concourse.mybir as mybir
    import concourse.tile as tile
    from concourse.masks import make_identity

    f32 = mybir.dt.float32
    f32r = mybir.dt.float32r
    bf16 = mybir.dt.bfloat16
    AF = mybir.ActivationFunctionType
    OP = mybir.AluOpType

    # Bacc (not raw Bass): its compile() runs move_matmul_waits_to_ldweights +
    # generate_event_semaphores, without which walrus rejects multi-wait matmuls
    nc = bacc.Bacc(None, target_bir_lowering=False, debug=False)

    pad_a = nc.declare_dram_parameter("pad_a", [2, P, 2500], bf16, isOutput=False)
    pad_b = nc.declare_dram_parameter("pad_b", [2, P, 2500], bf16, isOutput=False)
    wq = nc.declare_dram_parameter("wq", [2, P, 9 * P], bf16, isOutput=False)
    wk = nc.declare_dram_parameter("wk", [2, P, 9 * P], bf16, isOutput=False)
    wv = nc.declare_dram_parameter("wv", [2, P, 9 * P], bf16, isOutput=False)
    bias_d = nc.declare_dram_parameter("bias", [3, P, 1], f32, isOutput=False)
    out_d = nc.declare_dram_parameter("out", [P, T], f32, isOutput=True)

    with tile.TileContext(nc) as tc, ExitStack() as ctx:
        const = ctx.enter_context(tc.tile_pool(name="const", bufs=1))
        sb = ctx.enter_context(tc.tile_pool(name="sb", bufs=1))
        fin = ctx.enter_context(tc.tile_pool(name="fin", bufs=2))
        ep = ctx.enter_context(tc.tile_pool(name="ep", bufs=2))
        psumB = ctx.enter_context(tc.tile_pool(name="psum", bufs=2, space="PSUM"))

        identity = const.tile([P, P], bf16)
        make_identity(nc, identity)
        # maskP[k, m] = 1 iff k == 32*(m//32): K=128 broadcast matmul takes
        # the per-head reciprocal rows (at partitions 0/32/64/96) to all 128
        # output partitions in one shot.  Built on host (DVE memsets can't
        # target unaligned partition bases); DVE copy provides f32r rounding.
        maskP_d = nc.declare_dram_parameter("maskP", [P, P], f32, isOutput=False)
        maskPf = const.tile([P, P], f32)
        nc.sync.dma_start(maskPf[:], maskP_d[:])
        maskP = const.tile([P, P], f32r)
        nc.vector.tensor_copy(maskP[:], maskPf[:])  # f32 -> f32r rounding
        # reciprocal landing tile: rows 32h hold 1/denom for head h; all other
        # rows stay zero from this one-time clear (the mask zeroes them too)
        rb = sb.tile([P, NL], f32r, tag="rb")
        nc.vector.memset(rb[:].bitcast(f32), 0.0)

        # ---- input DMAs.  k weights + image-B band 0 gate the first matmul;
        # q weights + image-A band 0 gate the q tile (and with it the first
        # exp); v weights later (v-branch runs as loop filler).
        wk_sb = sb.tile([P, 2 * 9 * P], bf16, tag="wk")
        pb_sb = sb.tile([P, 2 * 2500], bf16, tag="pb")
        wv_sb = sb.tile([P, 2 * 9 * P], bf16, tag="wv")
        wq_sb = sb.tile([P, 2 * 9 * P], bf16, tag="wq")
        pa_sb = sb.tile([P, 2 * 2500], bf16, tag="pa")
        bias_sb = sb.tile([P, 3], f32, tag="bias")

        def _img_band(dst, src, kc, r0, r1):
            nc.sync.dma_start(dst[:, kc * 2500 + r0 * 50:kc * 2500 + r1 * 50],
                              src[kc][:, r0 * 50:r1 * 50])

        for kc in range(2):
            nc.sync.dma_start(wk_sb[:, kc * 1152:(kc + 1) * 1152], wk[kc])
        for kc in range(2):
            _img_band(pb_sb, pad_b, kc, *DMA_BANDS[0])
        for kc in range(2):
            nc.sync.dma_start(wq_sb[:, kc * 1152:(kc + 1) * 1152], wq[kc])
        for kc in range(2):
            _img_band(pa_sb, pad_a, kc, *DMA_BANDS[0])
        for r in range(3):
            nc.sync.dma_start(bias_sb[:, r:r + 1], bias_d[r])
        for kc in range(2):
            nc.sync.dma_start(wv_sb[:, kc * 1152:(kc + 1) * 1152], wv[kc])
        for i, (r0, r1) in enumerate(DMA_BANDS):
            if i > 0:
                for kc in range(2):
                    _img_band(pb_sb, pad_b, kc, r0, r1)
                for kc in range(2):
                    _img_band(pa_sb, pad_a, kc, r0, r1)

        qT = sb.tile([P, T], bf16, tag="qT")
        qTf = sb.tile([P, T], f32, tag="qTf")   # fp32 copy for the residual
        kT = sb.tile([P, T], bf16, tag="kT")
        vT = sb.tile([P, T], bf16, tag="vT")
        # per t-chunk, per head: 33 cols = [v(32) | ones(1)]; the ones column
        # carries the softmax denominator through the AV matmul, and LDWEIGHTS
        # cost scales with weight columns (33 beats 64)
        vaug = sb.tile([P, N_TCH * 132], bf16, tag="vaug")
        nc.vector.memset(vaug[:], 0.0)
        ones_cols = vaug.rearrange("p (j h c) -> p j h c", h=4, c=33)[:, :, :, 32:33]
        nc.vector.memset(ones_cols, 1.0)
        outbuf = sb.tile([P, T], f32, tag="outbuf")

        # block-diagonal expanded q: qTb[32h:32h+32, h*256:(h+1)*256] = q strip
        # for the current l-tile, zeros elsewhere (ping-pong by l-tile parity)
        qTb0 = sb.tile([P, 4 * NL], bf16, tag="qTb0")
        qTb1 = sb.tile([P, 4 * NL], bf16, tag="qTb1")
        qTb = [qTb0, qTb1]
        nc.vector.memset(qTb0[:], 0.0)
        nc.vector.memset(qTb1[:], 0.0)

        def build_qtb(li):
            dst = qTb[li % 2]
            for h in range(4):
                nc.vector.tensor_copy(
                    dst[32 * h:32 * h + 32, h * NL:(h + 1) * NL],
                    qT[32 * h:32 * h + 32, li * NL:(li + 1) * NL])



        # ---------------- emission helpers ----------------
        def branch_items(w_sb, img_sb, dest, role, dest2=None):
            """One branch as a list of closures: 18 matmuls + 1 drain per
            t-tile (so the scheduler can interleave at matmul granularity)."""
            items = []
            for (t0, nt, r0, nr) in T_TILES:
                cell = {}

                def mk_mm(t0, nt, r0, nr, kc, di, dj, mm, cell):
                    def run():
                        if mm == 0:
                            cell["ps"] = psumB.tile(
                                [P, nt], f32, tag="scratch", bufs=2,
                                name=f"br_{role}_{t0}")
                        pv = img_sb[:, kc * 2500:(kc + 1) * 2500].rearrange(
                            "p (r c) -> p r c", c=50)
                        w_ = w_sb[:, kc * 1152:(kc + 1) * 1152]
                        tap = di * 3 + dj
                        rhs = pv[:, r0 + di:r0 + di + nr, dj:dj + 48]
                        nc.tensor.matmul(cell["ps"][:],
                                         w_[:, tap * P:(tap + 1) * P], rhs,
                                         start=(mm == 0), stop=(mm == 17),
                                         skip_group_check=True)
                    return run

                mm = 0
                for kc in range(2):
                    for di in range(3):
                        for dj in range(3):
                            items.append(mk_mm(t0, nt, r0, nr, kc, di, dj, mm, cell))
                            mm += 1

                def mk_drain(t0, nt, cell):
                    def run():
                        nc.vector.tensor_scalar_add(
                            dest[:, t0:t0 + nt], cell["ps"][:],
                            bias_sb[:, role:role + 1])
                        if dest2 is not None:
                            nc.vector.tensor_scalar_add(
                                dest2[:, t0:t0 + nt], cell["ps"][:],
                                bias_sb[:, role:role + 1])
                    return run

                items.append(mk_drain(t0, nt, cell))
            return items

        def transpose_chunk(c):
            """vT chunk c -> vaug [t, d] blocks (+ ones col already set)."""
            tp = psumB.tile([P, P], bf16, tag="scratch", bufs=2, name=f"tp_{c}")
            nc.tensor.transpose(tp[:], vT[:, c * P:(c + 1) * P], identity[:])
            dst = vaug[:, c * 132:(c + 1) * 132].rearrange(
                "p (h c2) -> p h c2", c2=33)[:, :, 0:32]
            src = tp[:].rearrange("p (h c2) -> p h c2", c2=32)
            nc.vector.tensor_copy(dst, src)

        sc_t, et_t, av_t = {}, {}, {}

        def scores(j):
            li, ch = divmod(j, N_TCH)
            t = psumB.tile([P, 4 * NL], f32, tag="sc", bufs=2, name=f"sc_{j}")
            sc_t[j] = t
            # fp32 PSUM matmul output is capped at one bank (N=512): two MMs
            # of 2 heads each (same stationary weights).  rhs is qTb, a
            # per-l-tile block layout [q0|q1|q2|q3] built by 4 DVE copies.
            for half in range(2):
                nc.tensor.matmul(t[:, half * 512:(half + 1) * 512],
                                 kT[:, ch * P:(ch + 1) * P],
                                 qTb[li % 2][:, half * 512:(half + 1) * 512],
                                 start=True, stop=True)

        def do_exp(j):
            et = ep.tile([P, 4 * NL], bf16, tag="et", bufs=ET_BUFS,
                         name=f"et_{j}")
            et_t[j] = et
            nc.scalar.activation(et[:], sc_t.pop(j)[:], AF.Exp, scale=-0.0625)

        def av(j):
            li, ch = divmod(j, N_TCH)
            if ch == 0:
                av_t[li] = psumB.tile([P, 2 * NL], f32, tag="avout", bufs=2,
                                      name=f"avout_{li}")
            outp = av_t[li]
            et = et_t.pop(j)
            for h in range(4):
                cp, pbase = NL * (h // 2), 64 * (h % 2)
                nc.tensor.matmul(
                    outp[pbase:pbase + 33, cp:cp + NL],
                    vaug[:, ch * 132 + 33 * h:ch * 132 + 33 * h + 33],
                    et[:, h * NL:(h + 1) * NL],
                    start=(ch == 0), stop=(ch == N_TCH - 1),
                    tile_position=(0, 64 * (h % 2)), skip_group_check=True)

        fin_state = {}

        def fin_step(li, sub):
            outp = av_t[li]
            l0 = li * NL
            if sub == 0:
                # per-head reciprocals straight from PSUM into the 32-aligned
                # rows of rb (no staging, no DMAs)
                with nc.allow_low_precision(reason="f32r recip: 2^-19 rel "
                                            "is ample for softmax denom"):
                    for h in range(2):
                        cp, pbase = NL * (h // 2), 64 * (h % 2)
                        nc.vector.reciprocal(
                            rb[32 * h:32 * h + 1, :],
                            outp[pbase + 32:pbase + 33, cp:cp + NL])
            elif sub == 1:
                with nc.allow_low_precision(reason="f32r recip: 2^-19 rel "
                                            "is ample for softmax denom"):
                    for h in range(2, 4):
                        cp, pbase = NL * (h // 2), 64 * (h % 2)
                        nc.vector.reciprocal(
                            rb[32 * h:32 * h + 1, :],
                            outp[pbase + 32:pbase + 33, cp:cp + NL])
            elif sub == 2:
                bc_ps = psumB.tile([P, NL], f32, tag="scratch", bufs=2,
                                   name=f"bc_{li}")
                nc.tensor.matmul(bc_ps[:], maskP[:], rb[:],
                                 start=True, stop=True)
                bc_sb = fin.tile([P, NL], f32, tag="bcsb", bufs=2,
                                 name=f"bcsb_{li}")
                fin_state["bc_sb"] = bc_sb
                nc.vector.tensor_copy(bc_sb[:], bc_ps[:])
            elif sub == 3:
                bc_sb = fin_state["bc_sb"]
                for h in range(4):
                    cp, pbase = NL * (h // 2), 64 * (h % 2)
                    nc.vector.tensor_tensor(
                        outbuf[h * 32:(h + 1) * 32, l0:l0 + NL],
                        outp[pbase:pbase + 32, cp:cp + NL],
                        bc_sb[32 * h:32 * h + 32, :], op=OP.mult)
            elif sub == 4:
                # residual add: uniform across all 128 partitions -> one op
                nc.vector.tensor_tensor(
                    outbuf[:, l0:l0 + NL], outbuf[:, l0:l0 + NL],
                    qTf[:, l0:l0 + NL], op=OP.add)
            elif sub == 5:
                nc.sync.dma_start(out_d[:, l0:l0 + NL], outbuf[:, l0:l0 + NL])
                del av_t[li]

        # ---------------- upfront: k-branch || l-tile-0 attention ----------
        k_items = branch_items(wk_sb, pb_sb, kT, 1)
        q_items = branch_items(wq_sb, pa_sb, qT, 0, dest2=qTf)
        v_items = branch_items(wv_sb, pb_sb, vT, 2)

        KB = 19  # items per branch t-tile (18 matmuls + drain)
        for it in k_items[0:KB]:          # k t-tile 0
            it()
        for it in q_items[0:KB]:          # q t-tile 0 (covers l-tiles 0..1)
            it()
        build_qtb(0)
        for ti in range(5):               # k t-tiles 1-4 alternate with the
            if ti > 0:                    # l0 scores/exps they unlock
                for it in k_items[ti * KB:(ti + 1) * KB]:
                    it()
            for ch in range(*TILE_CHUNKS[ti]):
                scores(ch)
                do_exp(ch)
        for it in q_items[KB:2 * KB]:     # q t-tile 1 (l-tiles 1..3)
            it()
        build_qtb(1)
        scores(18)

        # ---------------- main loop: one step per exp ----------------------
        # fillers: v-branch (with transposes after each tile drain), then
        # q t-tiles 2-4.  AV is emission-gated on vaug availability.
        fillers = []
        v_done_chunks = [0]
        for ti in range(5):
            fillers.extend(v_items[ti * KB:(ti + 1) * KB])

            def mk_tp(ti):
                def run():
                    for c in range(*TILE_CHUNKS[ti]):
                        transpose_chunk(c)
                    v_done_chunks[0] = TILE_CHUNKS[ti][1]
                return run

            fillers.append(mk_tp(ti))
        q_done_end = [960]   # qT columns drained by emitted q-tiles (0,1 upfront)
        for ti in range(2, 5):
            fillers.extend(q_items[ti * KB:(ti + 1) * KB])

            def mk_qmark(end):
                def run():
                    q_done_end[0] = end
                return run

            fillers.append(mk_qmark(T_TILES[ti][0] + T_TILES[ti][1]))

        fin_queue = []   # (li, sub) pending finalize substeps
        av_next = [0]

        def try_avs(j, budget):
            n = 0
            while av_next[0] <= j - 2 and n < budget:
                jj = av_next[0]
                li_a, ch_a = divmod(jj, N_TCH)
                if ch_a >= v_done_chunks[0]:
                    break  # its vaug chunk is not emitted yet
                if ch_a == 0 and li_a >= 2:
                    # avout bufs=2: force finalize(li_a-2) fully emitted first
                    while fin_queue and fin_queue[0][0] <= li_a - 2:
                        fin_step(*fin_queue.pop(0))
                av(jj)
                if ch_a == N_TCH - 1:
                    fin_queue.extend((li_a, s) for s in range(6))
                av_next[0] += 1
                n += 1

        for j in range(N_TCH, NJ):
            li, jl = divmod(j, N_TCH)
            # PE fillers: 4/step while the v-branch is pending, 2 after
            nfill = 4 if v_done_chunks[0] < N_TCH else 2
            for _ in range(nfill):
                if fillers:
                    fillers.pop(0)()
            if jl == 15 and li + 1 < N_LT:
                # the build reads qT[:, (li+1)*256:(li+2)*256]: force-emit
                # filler items until those q-branch drains are in the stream
                # (DVE executes in emission order -- a late drain would make
                # the copy read uninitialized qT on the first dispatch)
                need = min(T, (li + 2) * NL)
                while fillers and q_done_end[0] < need:
                    fillers.pop(0)()
                build_qtb(li + 1)
            if j + 1 < NJ:
                scores(j + 1)
            do_exp(j)
            # keep AV within the et ring (lag < ET_BUFS-1), else force-drain
            while av_next[0] < j - (ET_BUFS - 3):
                if av_next[0] % N_TCH >= v_done_chunks[0]:
                    while fillers and av_next[0] % N_TCH >= v_done_chunks[0]:
                        fillers.pop(0)()
                try_avs(j, 1)
            try_avs(j, 2 if fillers else 3)
            # one finalize substep every OTHER step: gives the DVE chain
            # (den copies -> recip) runway so the bc matmul never
            # head-of-line-blocks the PE queue
            if fin_queue and j % 2 == 0:
                fin_step(*fin_queue.pop(0))

        # ---------------- tail ----------------
        while fillers:
            fillers.pop(0)()
        while av_next[0] < NJ:
            try_avs(NJ + 1, 4)
        while fin_queue:
            fin_step(*fin_queue.pop(0))

    nc.compile()
    return nc


def _fold_weights(dw_w, bn_gamma, bn_beta, bn_mean, bn_var, pw_w, pw_b, lin_w):
    """Fold BN + pointwise conv + linear (+ depthwise taps) per branch.

    Returns Wtap [6, 9, 256, 256] (float32) and bias c [6, 256]."""
    dw = dw_w.astype(np.float64)
    g = bn_gamma.astype(np.float64)
    b = bn_beta.astype(np.float64)
    m = bn_mean.astype(np.float64)
    v = bn_var.astype(np.float64)
    pw = pw_w.astype(np.float64)
    pb = pw_b.astype(np.float64)
    lw = lin_w.astype(np.float64)

    scale = g / np.sqrt(v + EPS)                      # [6, 256]
    shift = b - m * scale                             # [6, 256]
    M = np.einsum("noc,ncd->nod", lw, pw)             # lin @ pw  [6, 256, 256]
    W = M * scale[:, None, :]                         # [6, 256(o), 256(c)]
    c = np.einsum("noc,nc->no", M, shift) + np.einsum("noc,nc->no", lw, pb)
    # taps: Wtap[n, di*3+dj, o, c] = W[n, o, c] * dw[n, c, di, dj]
    Wtap = W[:, None, :, :] * dw.transpose(0, 2, 3, 1).reshape(6, 9, 1, 256)
    return Wtap.astype(np.float32), c.astype(np.float32)


def _bf16(a):
    import ml_dtypes
    return a.astype(ml_dtypes.bfloat16)


def _pad_images(x):
    """x [B, T, 256] -> per batch channel-major zero-padded bf16 [2,128,2500]."""
    out = np.zeros((B, 2, P, 50, 50), dtype=np.float32)
    img = np.ascontiguousarray(x.transpose(0, 2, 1)).reshape(B, DIM, HW, HW)
    out[:, :, :, 1:49, 1:49] = img.reshape(B, 2, P, HW, HW)
    return _bf16(out.reshape(B, 2, P, 2500))


def _wtap_lhsT(Wtap, branch, g):
    """Pack lhsT layout [2, 128, 9*128] for a branch restricted to quad g."""
    rows = slice(g * P, (g + 1) * P)
    out = np.empty((2, P, 9 * P), dtype=np.float32)
    for kc in range(2):
        for tap in range(9):
            blk = Wtap[branch, tap][rows, kc * P:(kc + 1) * P]  # [128 o, 128 c]
            out[kc, :, tap * P:(tap + 1) * P] = blk.T
    return _bf16(out)


def kernel(x1, x2, dw_w, bn_gamma, bn_beta, bn_mean, bn_var, pw_w, pw_b, lin_w,
           h1=HW, w1=HW, h2=HW, w2=HW):
    global _PROGRAM
    from concourse.bass_utils import run_bass_kernel_spmd

    x1 = np.asarray(x1, dtype=np.float32)
    x2 = np.asarray(x2, dtype=np.float32)

    Wtap, c = _fold_weights(np.asarray(dw_w), np.asarray(bn_gamma),
                            np.asarray(bn_beta), np.asarray(bn_mean),
                            np.asarray(bn_var), np.asarray(pw_w),
                            np.asarray(pw_b), np.asarray(lin_w))
    pad1 = _pad_images(x1)   # [B, 2, 128, 2500]
    pad2 = _pad_images(x2)
    maskP = np.zeros((P, P), dtype=np.float32)
    for m in range(P):
        maskP[32 * (m // 32), m] = 1.0

    if _PROGRAM is None:
        _PROGRAM = _build_program()
    nc = _PROGRAM

    # core layout: core = m*4 + b*2 + g
    # map m=0: o1 = att(q=br0(x1), k=br4(x2), v=br5(x2)) + q1
    # map m=1: o2 = att(q=br3(x2), k=br1(x1), v=br2(x1)) + q2
    in_maps = []
    for m in range(2):
        qbr, kbr, vbr = (0, 4, 5) if m == 0 else (3, 1, 2)
        pa, pb_ = (pad1, pad2) if m == 0 else (pad2, pad1)
        for b in range(2):
            for g in range(2):
                bias = np.stack([c[qbr, g * P:(g + 1) * P],
                                 c[kbr, g * P:(g + 1) * P],
                                 c[vbr, g * P:(g + 1) * P]])[:, :, None]
                in_maps.append({
                    "pad_a": np.ascontiguousarray(pa[b]),
                    "pad_b": np.ascontiguousarray(pb_[b]),
                    "wq": _wtap_lhsT(Wtap, qbr, g),
                    "wk": _wtap_lhsT(Wtap, kbr, g),
                    "wv": _wtap_lhsT(Wtap, vbr, g),
                    "bias": np.ascontiguousarray(bias),
                    "maskP": maskP,
                })

    global _last_in_maps
    _last_in_maps = in_maps
    res = run_bass_kernel_spmd(nc, in_maps, list(range(N_CORES)))

    o = np.empty((2, 2, HEADS, T, DH), dtype=np.float32)
    for m in range(2):
        for b in range(2):
            for g in range(2):
                core = m * 4 + b * 2 + g
                blk = res.results[core]["out"].reshape(4, DH, T)
                o[m, b, 4 * g:4 * g + 4] = blk.transpose(0, 2, 1)
    o1 = o[0].reshape(B, T, HEADS * DH)
    o2 = o[1].reshape(B, T, HEADS * DH)
    return o1, o2
